# revision 4
# baseline (speedup 1.0000x reference)
"""Trainium2 Bass kernel for nn_HGNN: 10-layer spectral hypergraph GNN.

Sharding: row-partition of the node dimension N=2048 across 8 cores (256
rows each). Per layer, each core computes z = (L @ x)[own rows] with L-column
tiles as stationary operands (z in normal layout -> Dirichlet trace energy is
a direct elementwise product against the previous activation), transposes z
on the PE to feed h = relu(z @ W.T), AllGathers h into the next full x, and
computes the edge-sharded expanded Dirichlet energy (m = Hs^T x) inside the
AllGather window of the following layer. Layer-9 scalar tails (C=40) and the
cross-core psums, log_softmax, W spectral norms, and the eigengap of L are
host-side.

Set HGNN_AG_BF16=0 to transport the AllGather in fp32 instead of bf16.
"""
import os
import numpy as np

N, E, F, C = 2048, 1024, 512, 40
LAYERS = 10
EPS = 1e-10
NCORES = 8
NS = N // NCORES          # 256 node rows per core
ES = E // NCORES          # 128 edges per core
KT = N // 128             # 16 k tiles
FT = F // 128             # 4 feature tiles
AG_BF16 = os.environ.get("HGNN_AG_BF16", "1") == "1"

_CACHE = {}


def _build():
    from concourse import bacc, tile, mybir

    F32 = mybir.dt.float32
    F32R = mybir.dt.float32r
    BF16 = mybir.dt.bfloat16
    SQUARE = mybir.ActivationFunctionType.Square
    AX = mybir.AxisListType.X
    TDT = BF16 if AG_BF16 else F32R  # AllGather transport dtype

    nc = bacc.Bacc("TRN2", target_bir_lowering=False, debug=False,
                   num_devices=NCORES)

    xin = nc.dram_tensor("xin", [N, F], F32R, kind="ExternalInput").ap()
    lcs = nc.dram_tensor("lcs", [N, NS], F32R, kind="ExternalInput").ap()
    hsd = nc.dram_tensor("hsd", [N, ES], F32R, kind="ExternalInput").ap()
    wts = nc.dram_tensor("wts", [(LAYERS - 1) * F, F], F32R, kind="ExternalInput").ap()
    wlast = nc.dram_tensor("wlast", [F, C], F32R, kind="ExternalInput").ap()
    ident = nc.dram_tensor("ident", [128, 128], F32, kind="ExternalInput").ap()
    cvec = nc.dram_tensor("cvec", [NS, 1], F32, kind="ExternalInput").ap()
    invde = nc.dram_tensor("invde", [ES, 1], F32, kind="ExternalInput").ap()

    x10 = nc.dram_tensor("x10", [NS, C], F32, kind="ExternalOutput").ap()
    part = nc.dram_tensor("part", [1, 48], F32, kind="ExternalOutput").ap()

    RG = [list(range(NCORES))]

    with tile.TileContext(nc) as tc:
        with tc.tile_pool(name="sbL", bufs=1) as sbL, \
             tc.tile_pool(name="sbX", bufs=1) as sbX, \
             tc.tile_pool(name="sbW", bufs=2) as sbW, \
             tc.tile_pool(name="sbZ", bufs=2) as sbZ, \
             tc.tile_pool(name="sbH", bufs=2) as sbH, \
             tc.tile_pool(name="sbS", bufs=2) as sbS, \
             tc.tile_pool(name="sbC", bufs=1) as sbC, \
             tc.tile_pool(name="psA", bufs=2, space="PSUM") as psA, \
             tc.tile_pool(name="psB", bufs=2, space="PSUM") as psB, \
             tc.tile_pool(name="psT", bufs=2, space="PSUM") as psT, \
             tc.tile_pool(name="psM", bufs=1, space="PSUM") as psM, \
             tc.tile_pool(name="dramP", bufs=2, space="DRAM") as dramP:

            # ---- resident loads ----
            lcs_sb = [sbL.tile([128, NS], F32R, tag=f"lcs{k}", name=f"lcs{k}")
                      for k in range(KT)]
            hs_sb = [sbL.tile([128, ES], F32R, tag=f"hs{k}", name=f"hs{k}")
                     for k in range(KT)]
            for k in range(KT):
                nc.sync.dma_start(lcs_sb[k][:], lcs[k * 128:(k + 1) * 128, :])
                nc.sync.dma_start(hs_sb[k][:], hsd[k * 128:(k + 1) * 128, :])

            xbuf = [[sbX.tile([128, F], F32R, tag=f"x{p}_{k}", name=f"x{p}_{k}")
                     for k in range(KT)] for p in range(2)]
            for k in range(KT):
                nc.sync.dma_start(xbuf[0][k][:], xin[k * 128:(k + 1) * 128, :])
            xrb = [sbX.tile([128, F], TDT, tag=f"xrb{k}", name=f"xrb{k}")
                   for k in range(KT)] if AG_BF16 else None

            id_sb = sbC.tile([128, 128], F32, tag="id", name="id")
            nc.sync.dma_start(id_sb[:], ident[:])
            ones_sb = sbC.tile([128, 1], F32, tag="ones", name="ones")
            nc.vector.memset(ones_sb[:], 1.0)
            ccol = [sbC.tile([128, 1], F32, tag=f"ccol{h}", name=f"ccol{h}")
                    for h in range(2)]
            nc.sync.dma_start(ccol[0][:], cvec[0:128, :])
            nc.sync.dma_start(ccol[1][:], cvec[128:256, :])
            invde_sb = sbC.tile([128, 1], F32, tag="invde", name="invde")
            nc.sync.dma_start(invde_sb[:], invde[:])
            acc = sbC.tile([1, 48], F32, tag="acc", name="acc")
            nc.vector.memset(acc[:], 0.0)

            h_prev = None  # own rows of x_i ([128,F] x2, f32r-viewable)

            for i in range(LAYERS):
                last = (i == LAYERS - 1)
                x_cur = xbuf[i % 2]

                # stream this layer's weight (transposed [F, out]) into SBUF
                OUTW = C if last else F
                wtl = [sbW.tile([128, OUTW], F32R, tag=f"w{t}", name=f"w{t}")
                       for t in range(FT)]
                for t in range(FT):
                    if last:
                        nc.sync.dma_start(wtl[t][:], wlast[t * 128:(t + 1) * 128, :])
                    else:
                        r0 = i * F + t * 128
                        nc.sync.dma_start(wtl[t][:], wts[r0:r0 + 128, :])

                # ---- step A: z[n'own, f] = sum_k L[k, n'] x[k, f] (normal) ----
                z_sb = [sbZ.tile([128, F], F32R, tag=f"z{mb}", name=f"z{mb}")
                        for mb in range(2)]
                st = sbS.tile([128, 8], F32, tag="st", name="st")
                for mb in range(2):
                    pa = psA.tile([128, F], F32, tag="pa", name="pa")
                    for k in range(KT):
                        nc.tensor.matmul(
                            pa[:], lcs_sb[k][:, mb * 128:(mb + 1) * 128],
                            x_cur[k][:], start=(k == 0), stop=(k == KT - 1))
                    nc.vector.tensor_copy(z_sb[mb][:], pa[:])
                    # energy[i-1] = sum over own rows of x_i * z_i
                    if i >= 1:
                        scrE = sbS.tile([128, F], F32, tag="scrE", name="scrE")
                        hp = h_prev[mb][:]
                        if hp.dtype == F32R:
                            hp = hp.bitcast(F32)
                        nc.vector.tensor_mul(scrE[:], z_sb[mb][:].bitcast(F32), hp)
                        nc.vector.reduce_sum(st[:, mb:mb + 1], scrE[:], axis=AX)

                # ---- transpose z -> zT tiles for step B ----
                ztl = [sbZ.tile([128, NS], F32R, tag=f"zt{ft}", name=f"zt{ft}")
                       for ft in range(FT)]
                for ft in range(FT):
                    for mb in range(2):
                        ptp = psT.tile([128, 128], F32, tag="ptp", name="ptp")
                        nc.tensor.transpose(
                            ptp[:], z_sb[mb][:, ft * 128:(ft + 1) * 128].bitcast(F32),
                            id_sb[:])
                        nc.vector.tensor_copy(
                            ztl[ft][:, mb * 128:(mb + 1) * 128], ptp[:])

                # ---- step B: h[n', o] = relu(sum_f zT[f, n'] Wt[f, o]) ----
                hdt = F32R if last else TDT
                hh = [sbH.tile([128, OUTW], hdt, tag=f"hh{nt}", name=f"hh{nt}")
                      for nt in range(2)]
                for nt in range(2):
                    pb = psB.tile([128, OUTW], F32, tag="pb", name="pb")
                    for fk in range(FT):
                        nc.tensor.matmul(
                            pb[:], ztl[fk][:, nt * 128:(nt + 1) * 128],
                            wtl[fk][:], start=(fk == 0), stop=(fk == FT - 1))
                    nc.vector.tensor_scalar_max(hh[nt][:], pb[:], 0.0)

                # ---- term1[i<9]: sum_n' c[n'] * sum_f h^2 (own rows) ----
                if not last:
                    for nt in range(2):
                        scrQ = sbS.tile([128, F], F32, tag="scrQ", name="scrQ")
                        hq = hh[nt][:]
                        if hdt == F32R:
                            hq = hq.bitcast(F32)
                        nc.scalar.activation(scrQ[:, 0:OUTW], hq, SQUARE)
                        rcol = sbS.tile([128, 1], F32, tag="rcol", name="rcol")
                        nc.vector.reduce_sum(rcol[:], scrQ[:, 0:OUTW], axis=AX)
                        nc.vector.tensor_mul(st[:, 2 + nt:3 + nt], rcol[:],
                                             ccol[nt][:])

                # ---- AllGather h -> x_{i+1} (layers 0..8) ----
                if not last:
                    agi = dramP.tile([NS, OUTW], TDT, tag="agi", name="agi")
                    ago = dramP.tile([N, OUTW], TDT, tag="ago", name="ago",
                                     addr_space="Shared")
                    for nt in range(2):
                        nc.sync.dma_start(agi[nt * 128:(nt + 1) * 128, :], hh[nt][:])
                    nc.gpsimd.collective_compute(
                        "AllGather", mybir.AluOpType.bypass, replica_groups=RG,
                        ins=[agi.opt()], outs=[ago.opt()])
                    x_next = xbuf[(i + 1) % 2]
                    for k in range(KT):
                        if AG_BF16:
                            nc.sync.dma_start(xrb[k][:], ago[k * 128:(k + 1) * 128, :])
                            nc.vector.tensor_copy(x_next[k][:], xrb[k][:])
                        else:
                            nc.sync.dma_start(x_next[k][:], ago[k * 128:(k + 1) * 128, :])
                else:
                    for nt in range(2):
                        nc.sync.dma_start(
                            x10[nt * 128:(nt + 1) * 128, :], hh[nt][:].bitcast(F32))

                # ---- term2[i-1] (fills this layer's AG window): m = Hs^T x_i ----
                if i >= 1:
                    pm = psM.tile([128, F], F32, tag="pm", name="pm")
                    for k in range(KT):
                        nc.tensor.matmul(pm[:], hs_sb[k][:], x_cur[k][:],
                                         start=(k == 0), stop=(k == KT - 1))
                    scrM = sbS.tile([128, F], F32, tag="scrM", name="scrM")
                    nc.scalar.activation(scrM[:], pm[:], SQUARE)
                    mcol = sbS.tile([128, 1], F32, tag="mcol", name="mcol")
                    nc.vector.reduce_sum(mcol[:], scrM[:], axis=AX)
                    nc.vector.tensor_mul(st[:, 4:5], mcol[:], invde_sb[:])

                # ---- harvest: partition-reduce st, then segment sums ----
                pss = psM.tile([1, 8], F32, tag="pss", name="pss")
                nc.tensor.matmul(pss[:], ones_sb[:], st[:], start=True, stop=True)
                scrow = sbS.tile([1, 8], F32, tag="scrow", name="scrow")
                nc.vector.tensor_copy(scrow[:], pss[:])
                if i >= 1:
                    nc.vector.reduce_sum(acc[0:1, i - 1:i], scrow[0:1, 0:2], axis=AX)
                    nc.vector.tensor_copy(acc[0:1, 32 + i - 1:33 + i - 1],
                                          scrow[0:1, 4:5])
                if not last:
                    nc.vector.reduce_sum(acc[0:1, 16 + i:17 + i], scrow[0:1, 2:4],
                                         axis=AX)

                h_prev = hh

            nc.sync.dma_start(part[:], acc[:])

    nc.compile()
    return nc


def _get_nc():
    if "nc" not in _CACHE:
        _CACHE["nc"] = _build()
    return _CACHE["nc"]


def kernel(x, L, H, W_first, W_mid, W_last):
    from concourse.bass_utils import run_bass_kernel_spmd

    x = np.asarray(x, dtype=np.float32)
    L = np.asarray(L, dtype=np.float32)
    H = np.asarray(H, dtype=np.float32)
    W_first = np.asarray(W_first, dtype=np.float32)
    W_mid = np.asarray(W_mid, dtype=np.float32)
    W_last = np.asarray(W_last, dtype=np.float32)

    L2d = L[0]
    x0 = np.ascontiguousarray(x[0])

    d_v = H.sum(axis=1)
    d_e = H.sum(axis=0)
    dvs = 1.0 / np.sqrt(d_v + EPS)
    Hs = (H * dvs[:, None]).astype(np.float32)
    inv_de = (1.0 / (d_e + EPS)).astype(np.float32)
    w_n = H @ (d_e * inv_de)
    cfull = (w_n * dvs * dvs).astype(np.float32)

    wts = np.concatenate(
        [W_first.T] + [W_mid[i].T for i in range(LAYERS - 2)], axis=0)
    wts = np.ascontiguousarray(wts.astype(np.float32))
    wlast = np.ascontiguousarray(W_last.T.astype(np.float32))
    ident = np.eye(128, dtype=np.float32)

    nc = _get_nc()
    in_maps = []
    for c in range(NCORES):
        in_maps.append({
            "xin": x0,
            "lcs": np.ascontiguousarray(L2d[:, c * NS:(c + 1) * NS]),
            "hsd": np.ascontiguousarray(Hs[:, c * ES:(c + 1) * ES]),
            "wts": wts,
            "wlast": wlast,
            "ident": ident,
            "cvec": np.ascontiguousarray(cfull[c * NS:(c + 1) * NS, None]),
            "invde": np.ascontiguousarray(inv_de[c * ES:(c + 1) * ES, None]),
        })
    res = run_bass_kernel_spmd(nc, in_maps, core_ids=list(range(NCORES)))
    _CACHE["last_results"] = res

    xf = np.concatenate([res.results[c]["x10"] for c in range(NCORES)], axis=0)
    parts = np.stack([res.results[c]["part"][0] for c in range(NCORES)])
    psum = parts.sum(axis=0)
    energies = np.empty(LAYERS, dtype=np.float64)
    energies[0:9] = psum[0:9]
    term1 = np.empty(LAYERS, dtype=np.float64)
    term1[0:9] = psum[16:25]
    term2 = np.empty(LAYERS, dtype=np.float64)
    term2[0:9] = psum[32:41]

    # layer-9 scalars on host from x10 (tiny C=40 tail)
    xf64 = xf.astype(np.float64)
    energies[9] = float(np.sum(xf64 * (L2d.astype(np.float64) @ xf64)))
    m10 = Hs.astype(np.float64).T @ xf64
    term2[9] = float((inv_de.astype(np.float64) * (m10 * m10).sum(axis=1)).sum())
    term1[9] = float((cfull.astype(np.float64) * (xf64 * xf64).sum(axis=1)).sum())

    energies = energies.astype(np.float32)
    energies_exp = (term1 - term2).astype(np.float32)

    # log_softmax over the node axis (axis=1 of [1, N, C])
    mx = xf64.max(axis=0, keepdims=True)
    lse = np.log(np.exp(xf64 - mx).sum(axis=0, keepdims=True)) + mx
    log_probs = (xf64 - lse)[None].astype(np.float32)

    # spectral norms of the weights (largest singular value)
    weights = [W_first] + [W_mid[i] for i in range(LAYERS - 2)] + [W_last]
    snorms = np.array(
        [np.linalg.svd(w.astype(np.float64), compute_uv=False)[0] for w in weights],
        dtype=np.float32)

    # eigengap of L
    ev = np.linalg.eigvalsh(L2d.astype(np.float64))
    pos = ev[ev > 1e-10]
    gap = np.float32(pos.min()) if pos.size else np.float32(np.inf)

    return (log_probs, energies, energies_exp, snorms, gap)
